# revision 32
# baseline (speedup 1.0000x reference)
"""Trainium2 Bass kernel for Llama4TextExperts-style grouped MoE FFN (SwiGLU).

Full-input contract: kernel(**inputs) takes the complete unsharded tensors and
returns the full [4096, 1024] output. Internally: expert-parallel across the 8
NeuronCores — core e gets expert e's three weight matrices and that expert's
512-token group (tokens arrive pre-sorted by expert with equal group sizes).
All routing / transposition is done host-side in numpy; no collectives needed.

Per-core device program (three GEMMs, ~6.4 GFLOP):
  phase 1: gate^T = Wg^T-stationary @ x^T, up^T likewise; SwiGLU fused on
           ACT (Silu) + DVE (mul) into h^T [I, T] bf16 resident in SBUF.
  phase 2: y = h @ Wd with h^T slices stationary, Wd streaming from its
           natural [I, H] DRAM layout; y lands untransposed in PSUM.
"""

import numpy as np
import ml_dtypes

import concourse.bass as bass
import concourse.mybir as mybir
import concourse.tile as tile
from concourse.tile import add_dep_helper
from concourse import bacc
from concourse.bass_utils import run_bass_kernel_spmd

# Problem shape (hardcoded per contract)
E = 8          # experts == cores
T = 512        # tokens per expert group
H = 1024       # hidden
I = 2048       # intermediate
P = 128        # partitions
KT = H // P    # 8  k-tiles over hidden
IT = I // P    # 16 i-tiles over intermediate
WB = 4         # i-blocks of 512 columns for gate/up weight streaming
MT = T // P    # 4  token tiles

BF16 = mybir.dt.bfloat16
F32 = mybir.dt.float32

_compiled = None  # (nc, ) cached across calls


def _build():
    nc = bacc.Bacc(None)
    xT_d = nc.declare_dram_parameter("xT", [H, T], BF16, isOutput=False)
    wg_d = nc.declare_dram_parameter("wg", [H, I], BF16, isOutput=False)
    wu_d = nc.declare_dram_parameter("wu", [H, I], BF16, isOutput=False)
    wd_d = nc.declare_dram_parameter("wd", [I, H], BF16, isOutput=False)
    y_d = nc.declare_dram_parameter("y", [T, H], F32, isOutput=True)

    xT_r = xT_d.rearrange("(ko p) t -> p ko t", p=P)     # [128, 8, 512]
    wg_r = wg_d.rearrange("(ko p) i -> p ko i", p=P)     # [128, 8, 2048]
    wu_r = wu_d.rearrange("(ko p) i -> p ko i", p=P)
    wd_r = wd_d.rearrange("(io p) h -> p io h", p=P)     # [128, 16, 1024]

    with tile.TileContext(nc) as tc:
        with (
            tc.tile_pool(name="xpool", bufs=1) as xpool,
            tc.tile_pool(name="wdpool", bufs=1) as wdpool,
            tc.tile_pool(name="hpool", bufs=1) as hpool,
            tc.tile_pool(name="wpool", bufs=3) as wpool,
            tc.tile_pool(name="spool", bufs=3) as spool,
            tc.tile_pool(name="psum", bufs=2, space="PSUM") as psum,
        ):
            # Startup-critical loads. The DMA engines round-robin across all
            # outstanding descriptors (everything in flight completes
            # together at ~320GB/s aggregate), so later weight loads are
            # GATED on earlier DMA completions: the pipe first carries only
            # the ~1.5MB the first i-tile needs, then stays about one
            # compute-block ahead.
            # PE warm-up: the HAM clock gate holds the PE at 1.2GHz until it
            # has been busy ~3.4us. Run dummy matmuls on zeroed scratch SBUF
            # during the initial DMA wait so the real matmuls start at 2.4GHz.
            warm = xpool.tile([P, 512], BF16, tag="warm", name="warm")
            nc.gpsimd.memset(warm[:], 0.0)
            pwarm = psum.tile([P, 512], F32, tag="pg", name="pwarm")
            for _ in range(10):
                nc.tensor.matmul(
                    pwarm[:], warm[:, 0:128], warm[:], start=True, stop=True
                )

            wg_it = []   # per-i-tile gate weights for block 0
            wu_it = []
            d_stage = []  # last DMA instruction of each stage, for gating
            wg_it.append(xpool.tile([P, KT, 128], BF16, tag="wg_it0", name="wg_it0"))
            d0g = nc.sync.dma_start(wg_it[0][:], wg_r[:, :, 0:128])
            wu_it.append(xpool.tile([P, KT, 128], BF16, tag="wu_it0", name="wu_it0"))
            d0u = nc.sync.dma_start(wu_it[0][:], wu_r[:, :, 0:128])

            xk4 = []
            for kq in range(4):
                xc = xpool.tile([P, 2, T], BF16, tag=f"x{kq}", name=f"x{kq}")
                # alternate x^T chunks between the scalar HWDGE queue and the
                # gpsimd SWDGE queue: more descriptors in flight early while
                # the issue rate (~0.65us per descriptor per queue) ramps
                eng = nc.scalar if kq % 2 == 0 else nc.gpsimd
                eng.dma_start(xc[:], xT_r[:, 2 * kq:2 * kq + 2, :])
                xk4.append(xc)

            def xk(kt):
                return xk4[kt // 2][:, kt % 2, :]

            hT_sb = hpool.tile([P, IT, T], BF16)
            wd_sb = wdpool.tile([P, IT, H], BF16)

            def gated(dma_call, stage_idx, dep=None):
                if dep is not None:
                    add_dep_helper(
                        dma_call.ins, dep.ins, reason="dma staging throttle"
                    )
                elif stage_idx >= 0:
                    add_dep_helper(
                        dma_call.ins, d_stage[stage_idx].ins,
                        reason="dma staging throttle",
                    )
                return dma_call

            # stage A (ungated, with xT): it1 of block 0
            wg_it.append(xpool.tile([P, KT, 128], BF16, tag="wg_it1", name="wg_it1"))
            gated(nc.sync.dma_start(wg_it[1][:], wg_r[:, :, 128:256]), -1)
            wu_it.append(xpool.tile([P, KT, 128], BF16, tag="wu_it1", name="wu_it1"))
            d_stage.append(
                gated(nc.sync.dma_start(wu_it[1][:], wu_r[:, :, 128:256]), -1)
            )
            # it2+it3 of block 0, gated on the first it0 load (lookahead
            # gating keeps the pipe from draining dry between stages)
            wg_b0r = xpool.tile([P, KT, 256], BF16, tag="wg_b0r")
            gated(nc.sync.dma_start(wg_b0r[:], wg_r[:, :, 256:512]), -2, d0g)
            wu_b0r = xpool.tile([P, KT, 256], BF16, tag="wu_b0r")
            d_stage.append(
                gated(nc.sync.dma_start(wu_b0r[:], wu_r[:, :, 256:512]), -2, d0g)
            )
            # blocks 1..3: two-stage lookahead gating — enough in flight to
            # absorb HBM-bandwidth jitter, while stage A still gets the pipe
            # mostly to itself at the start
            wgf, wuf = {}, {}
            for wb in range(1, WB):
                wgb = wpool.tile([P, KT, 512], BF16, tag="wgf")
                gated(
                    nc.sync.dma_start(
                        wgb[:], wg_r[:, :, wb * 512:(wb + 1) * 512]
                    ),
                    max(wb - 2, 0),
                )
                wgf[wb] = wgb
                wub = wpool.tile([P, KT, 512], BF16, tag="wuf")
                d_stage.append(
                    gated(
                        nc.sync.dma_start(
                            wub[:], wu_r[:, :, wb * 512:(wb + 1) * 512]
                        ),
                        max(wb - 2, 0),
                    )
                )
                wuf[wb] = wub
            # Wd halves, chained behind the weight stream
            d_stage.append(
                gated(
                    nc.sync.dma_start(wd_sb[:, 0:8, :], wd_r[:, 0:8, :]), 2
                )
            )
            gated(nc.sync.dma_start(wd_sb[:, 8:16, :], wd_r[:, 8:16, :]), 3)

            def gu_slice(wb, itl, which):
                if wb == 0:
                    if itl < 2:
                        t = wg_it[itl] if which == "g" else wu_it[itl]
                        return lambda kt: t[:, kt, :]
                    t = wg_b0r if which == "g" else wu_b0r
                    return lambda kt: t[:, kt, (itl - 2) * P:(itl - 1) * P]
                t = wgf[wb] if which == "g" else wuf[wb]
                return lambda kt: t[:, kt, itl * P:(itl + 1) * P]

            for wb in range(WB):
                for itl in range(4):
                    it = wb * 4 + itl
                    gsl = gu_slice(wb, itl, "g")
                    usl = gu_slice(wb, itl, "u")
                    pg = psum.tile([P, T], F32, tag="pg")
                    pu = psum.tile([P, T], F32, tag="pu")
                    for kt in range(KT):
                        nc.tensor.matmul(
                            pg[:], gsl(kt), xk(kt),
                            start=(kt == 0), stop=(kt == KT - 1),
                        )
                    for kt in range(KT):
                        nc.tensor.matmul(
                            pu[:], usl(kt), xk(kt),
                            start=(kt == 0), stop=(kt == KT - 1),
                        )
                    sg = spool.tile([P, T], F32)
                    nc.scalar.activation(
                        sg[:], pg[:], mybir.ActivationFunctionType.Silu
                    )
                    nc.vector.tensor_mul(hT_sb[:, it, :], sg[:], pu[:])

            for mt in range(MT):
                ms = slice(mt * P, (mt + 1) * P)
                if mt < MT - 1:
                    py0 = psum.tile([P, 512], F32, tag="py0")
                    py1 = psum.tile([P, 512], F32, tag="py1")
                    for it in range(IT):
                        lhsT = hT_sb[:, it, ms]
                        nc.tensor.matmul(
                            py0[:], lhsT, wd_sb[:, it, 0:512],
                            start=(it == 0), stop=(it == IT - 1),
                        )
                        nc.tensor.matmul(
                            py1[:], lhsT, wd_sb[:, it, 512:1024],
                            start=(it == 0), stop=(it == IT - 1),
                        )
                    y0 = spool.tile([P, 512], F32, tag="y0")
                    nc.scalar.copy(y0[:], py0[:])
                    nc.sync.dma_start(y_d[ms, 0:512], y0[:])
                    y1 = spool.tile([P, 512], F32, tag="y1")
                    nc.vector.tensor_copy(y1[:], py1[:])
                    nc.sync.dma_start(y_d[ms, 512:1024], y1[:])
                else:
                    # last token tile: run the two 16-matmul chains
                    # back-to-back instead of interleaved, so the first
                    # half's copy+DMA overlaps the second half's matmuls and
                    # only one [128,512] copy+DMA remains after the last MM.
                    py0 = psum.tile([P, 512], F32, tag="py0")
                    for it in range(IT):
                        nc.tensor.matmul(
                            py0[:], hT_sb[:, it, ms], wd_sb[:, it, 0:512],
                            start=(it == 0), stop=(it == IT - 1),
                        )
                    y0 = spool.tile([P, 512], F32, tag="y0")
                    nc.scalar.copy(y0[:], py0[:])
                    nc.sync.dma_start(y_d[ms, 0:512], y0[:])
                    # split the remaining 512 columns into two N=256 chains
                    # (same total PE cycles) so the final exposed copy+DMA
                    # after the very last matmul is only [128,256]
                    py1a = psum.tile([P, 256], F32, tag="py1", name="py1a")
                    for it in range(IT):
                        nc.tensor.matmul(
                            py1a[:], hT_sb[:, it, ms], wd_sb[:, it, 512:768],
                            start=(it == 0), stop=(it == IT - 1),
                        )
                    y1 = spool.tile([P, 256], F32, tag="y1")
                    nc.scalar.copy(y1[:], py1a[:])
                    nc.sync.dma_start(y_d[ms, 512:768], y1[:])
                    # last two chains at N=128 (same total PE cycles as one
                    # N=256 chain): only a [128,128] copy + 64KB DMA remain
                    # exposed after the very last matmul
                    py1b = psum.tile([P, 128], F32, tag="pu", name="py1b")
                    for it in range(IT):
                        nc.tensor.matmul(
                            py1b[:], hT_sb[:, it, ms], wd_sb[:, it, 768:896],
                            start=(it == 0), stop=(it == IT - 1),
                        )
                    y2 = spool.tile([P, 128], F32, tag="y2")
                    nc.scalar.copy(y2[:], py1b[:])
                    nc.sync.dma_start(y_d[ms, 768:896], y2[:])
                    py1c = psum.tile([P, 128], F32, tag="py1", name="py1c")
                    for it in range(IT):
                        nc.tensor.matmul(
                            py1c[:], hT_sb[:, it, ms], wd_sb[:, it, 896:1024],
                            start=(it == 0), stop=(it == IT - 1),
                        )
                    y3 = spool.tile([P, 128], F32, tag="y3")
                    nc.vector.tensor_copy(y3[:], py1c[:])
                    nc.sync.dma_start(y_d[ms, 896:1024], y3[:])

    nc.compile()
    return nc


def _get_compiled():
    global _compiled
    if _compiled is None:
        _compiled = _build()
    return _compiled


def _numpy_fallback(hidden_states, gate_kernel, up_kernel, down_kernel, group_sizes):
    # Exact reference math on host; only used for unexpected group_sizes.
    out = np.empty((hidden_states.shape[0], down_kernel.shape[2]), np.float32)
    start = 0
    for e in range(gate_kernel.shape[0]):
        g = int(group_sizes[e])
        x = hidden_states[start:start + g]
        gate = x @ gate_kernel[e]
        up = x @ up_kernel[e]
        sig = np.where(
            gate >= 0,
            1.0 / (1.0 + np.exp(-np.clip(gate, 0, None))),
            np.exp(np.clip(gate, None, 0))
            / (1.0 + np.exp(np.clip(gate, None, 0))),
        )
        h = gate * sig * up
        out[start:start + g] = h @ down_kernel[e]
        start += g
    out[start:] = 0.0
    return out


def _make_in_maps(hidden_states, gate_kernel, up_kernel, down_kernel):
    bf = ml_dtypes.bfloat16
    in_maps = []
    for e in range(E):
        x_e = hidden_states[e * T:(e + 1) * T]
        in_maps.append({
            "xT": np.ascontiguousarray(x_e.T).astype(bf),
            "wg": np.ascontiguousarray(gate_kernel[e]).astype(bf),
            "wu": np.ascontiguousarray(up_kernel[e]).astype(bf),
            "wd": np.ascontiguousarray(down_kernel[e]).astype(bf),
        })
    return in_maps


def profile_run(inputs, tmpdir=None):
    """Dev helper (not used by grading): run with NTFF tracing, return exec ns."""
    nc = _get_compiled()
    in_maps = _make_in_maps(
        np.asarray(inputs["hidden_states"], np.float32),
        np.asarray(inputs["gate_kernel"], np.float32),
        np.asarray(inputs["up_kernel"], np.float32),
        np.asarray(inputs["down_kernel"], np.float32),
    )
    res = run_bass_kernel_spmd(
        nc, in_maps, core_ids=list(range(E)), trace=True, tmpdir=tmpdir
    )
    return res.exec_time_ns


def kernel(hidden_states, gate_kernel, up_kernel, down_kernel, group_sizes):
    hidden_states = np.asarray(hidden_states, dtype=np.float32)
    gate_kernel = np.asarray(gate_kernel, dtype=np.float32)
    up_kernel = np.asarray(up_kernel, dtype=np.float32)
    down_kernel = np.asarray(down_kernel, dtype=np.float32)
    gs = np.asarray(group_sizes)

    if not (gs.shape == (E,) and np.all(gs == T)):
        return _numpy_fallback(
            hidden_states, gate_kernel, up_kernel, down_kernel, gs
        )

    nc = _get_compiled()
    in_maps = _make_in_maps(hidden_states, gate_kernel, up_kernel, down_kernel)
    res = run_bass_kernel_spmd(nc, in_maps, core_ids=list(range(E)))
    return np.concatenate([res.results[e]["y"] for e in range(E)], axis=0)


# revision 33
# speedup vs baseline: 1.0385x; 1.0385x over previous
"""Trainium2 Bass kernel for Llama4TextExperts-style grouped MoE FFN (SwiGLU).

Full-input contract: kernel(**inputs) takes the complete unsharded tensors and
returns the full [4096, 1024] output. Internally: expert-parallel across the 8
NeuronCores — core e gets expert e's three weight matrices and that expert's
512-token group (tokens arrive pre-sorted by expert with equal group sizes).
All routing / transposition is done host-side in numpy; no collectives needed.

Per-core device program (three GEMMs, ~6.4 GFLOP):
  phase 1: gate^T = Wg^T-stationary @ x^T, up^T likewise; SwiGLU fused on
           ACT (Silu) + DVE (mul) into h^T [I, T] bf16 resident in SBUF.
  phase 2: y = h @ Wd with h^T slices stationary, Wd streaming from its
           natural [I, H] DRAM layout; y lands untransposed in PSUM.
"""

import numpy as np
import ml_dtypes

import concourse.bass as bass
import concourse.mybir as mybir
import concourse.tile as tile
from concourse.tile import add_dep_helper
from concourse import bacc
from concourse.bass_utils import run_bass_kernel_spmd

# Problem shape (hardcoded per contract)
E = 8          # experts == cores
T = 512        # tokens per expert group
H = 1024       # hidden
I = 2048       # intermediate
P = 128        # partitions
KT = H // P    # 8  k-tiles over hidden
IT = I // P    # 16 i-tiles over intermediate
WB = 4         # i-blocks of 512 columns for gate/up weight streaming
MT = T // P    # 4  token tiles

BF16 = mybir.dt.bfloat16
F32 = mybir.dt.float32

_compiled = None  # (nc, ) cached across calls


def _build():
    nc = bacc.Bacc(None)
    xT_d = nc.declare_dram_parameter("xT", [H, T], BF16, isOutput=False)
    wg_d = nc.declare_dram_parameter("wg", [H, I], BF16, isOutput=False)
    wu_d = nc.declare_dram_parameter("wu", [H, I], BF16, isOutput=False)
    wd_d = nc.declare_dram_parameter("wd", [I, H], BF16, isOutput=False)
    y_d = nc.declare_dram_parameter("y", [T, H], F32, isOutput=True)

    xT_r = xT_d.rearrange("(ko p) t -> p ko t", p=P)     # [128, 8, 512]
    wg_r = wg_d.rearrange("(ko p) i -> p ko i", p=P)     # [128, 8, 2048]
    wu_r = wu_d.rearrange("(ko p) i -> p ko i", p=P)
    wd_r = wd_d.rearrange("(io p) h -> p io h", p=P)     # [128, 16, 1024]

    with tile.TileContext(nc) as tc:
        with (
            tc.tile_pool(name="xpool", bufs=1) as xpool,
            tc.tile_pool(name="wdpool", bufs=1) as wdpool,
            tc.tile_pool(name="hpool", bufs=1) as hpool,
            tc.tile_pool(name="wpool", bufs=3) as wpool,
            tc.tile_pool(name="spool", bufs=3) as spool,
            tc.tile_pool(name="psum", bufs=2, space="PSUM") as psum,
        ):
            # Startup-critical loads. The DMA engines round-robin across all
            # outstanding descriptors (everything in flight completes
            # together at ~320GB/s aggregate), so later weight loads are
            # GATED on earlier DMA completions: the pipe first carries only
            # the ~1.5MB the first i-tile needs, then stays about one
            # compute-block ahead.
            # PE warm-up: the HAM clock gate holds the PE at 1.2GHz until it
            # has been busy ~3.4us. Run dummy matmuls on zeroed scratch SBUF
            # during the initial DMA wait so the real matmuls start at 2.4GHz.
            warm = xpool.tile([P, 512], BF16, tag="warm", name="warm")
            nc.gpsimd.memset(warm[:], 0.0)
            pwarm = psum.tile([P, 512], F32, tag="pg", name="pwarm")
            for _ in range(10):
                nc.tensor.matmul(
                    pwarm[:], warm[:, 0:128], warm[:], start=True, stop=True
                )

            wg_it = []   # per-i-tile gate weights for block 0
            wu_it = []
            d_stage = []  # last DMA instruction of each stage, for gating
            wg_it.append(xpool.tile([P, KT, 128], BF16, tag="wg_it0", name="wg_it0"))
            d0g = nc.sync.dma_start(wg_it[0][:], wg_r[:, :, 0:128])
            wu_it.append(xpool.tile([P, KT, 128], BF16, tag="wu_it0", name="wu_it0"))
            d0u = nc.sync.dma_start(wu_it[0][:], wu_r[:, :, 0:128])

            xk4 = []
            for kq in range(4):
                xc = xpool.tile([P, 2, T], BF16, tag=f"x{kq}", name=f"x{kq}")
                # alternate x^T chunks between the scalar HWDGE queue and the
                # gpsimd SWDGE queue: more descriptors in flight early while
                # the issue rate (~0.65us per descriptor per queue) ramps
                eng = nc.scalar if kq % 2 == 0 else nc.gpsimd
                eng.dma_start(xc[:], xT_r[:, 2 * kq:2 * kq + 2, :])
                xk4.append(xc)

            def xk(kt):
                return xk4[kt // 2][:, kt % 2, :]

            hT_sb = hpool.tile([P, IT, T], BF16)
            wd_sb = wdpool.tile([P, IT, H], BF16)

            def gated(dma_call, stage_idx, dep=None):
                if dep is not None:
                    add_dep_helper(
                        dma_call.ins, dep.ins, reason="dma staging throttle"
                    )
                elif stage_idx >= 0:
                    add_dep_helper(
                        dma_call.ins, d_stage[stage_idx].ins,
                        reason="dma staging throttle",
                    )
                return dma_call

            # stage A (ungated, with xT): it1 of block 0
            wg_it.append(xpool.tile([P, KT, 128], BF16, tag="wg_it1", name="wg_it1"))
            gated(nc.sync.dma_start(wg_it[1][:], wg_r[:, :, 128:256]), -1)
            wu_it.append(xpool.tile([P, KT, 128], BF16, tag="wu_it1", name="wu_it1"))
            d_stage.append(
                gated(nc.sync.dma_start(wu_it[1][:], wu_r[:, :, 128:256]), -1)
            )
            # it2+it3 of block 0, gated on the first it0 load (lookahead
            # gating keeps the pipe from draining dry between stages)
            wg_b0r = xpool.tile([P, KT, 256], BF16, tag="wg_b0r")
            gated(nc.sync.dma_start(wg_b0r[:], wg_r[:, :, 256:512]), -2, d0g)
            wu_b0r = xpool.tile([P, KT, 256], BF16, tag="wu_b0r")
            d_stage.append(
                gated(nc.sync.dma_start(wu_b0r[:], wu_r[:, :, 256:512]), -2, d0g)
            )
            # blocks 1..3: two-stage lookahead gating — enough in flight to
            # absorb HBM-bandwidth jitter, while stage A still gets the pipe
            # mostly to itself at the start
            wgf, wuf = {}, {}
            for wb in range(1, WB):
                wgb = wpool.tile([P, KT, 512], BF16, tag="wgf")
                gated(
                    nc.sync.dma_start(
                        wgb[:], wg_r[:, :, wb * 512:(wb + 1) * 512]
                    ),
                    max(wb - 2, 0),
                )
                wgf[wb] = wgb
                wub = wpool.tile([P, KT, 512], BF16, tag="wuf")
                d_stage.append(
                    gated(
                        nc.sync.dma_start(
                            wub[:], wu_r[:, :, wb * 512:(wb + 1) * 512]
                        ),
                        max(wb - 2, 0),
                    )
                )
                wuf[wb] = wub
            # Wd halves, chained behind the weight stream
            d_stage.append(
                gated(
                    nc.sync.dma_start(wd_sb[:, 0:8, :], wd_r[:, 0:8, :]), 2
                )
            )
            gated(nc.sync.dma_start(wd_sb[:, 8:16, :], wd_r[:, 8:16, :]), 3)

            def gu_slice(wb, itl, which):
                if wb == 0:
                    if itl < 2:
                        t = wg_it[itl] if which == "g" else wu_it[itl]
                        return lambda kt: t[:, kt, :]
                    t = wg_b0r if which == "g" else wu_b0r
                    return lambda kt: t[:, kt, (itl - 2) * P:(itl - 1) * P]
                t = wgf[wb] if which == "g" else wuf[wb]
                return lambda kt: t[:, kt, itl * P:(itl + 1) * P]

            for wb in range(WB):
                for itl in range(4):
                    it = wb * 4 + itl
                    gsl = gu_slice(wb, itl, "g")
                    usl = gu_slice(wb, itl, "u")
                    pg = psum.tile([P, T], F32, tag="pg")
                    pu = psum.tile([P, T], F32, tag="pu")
                    for kt in range(KT):
                        nc.tensor.matmul(
                            pg[:], gsl(kt), xk(kt),
                            start=(kt == 0), stop=(kt == KT - 1),
                        )
                    for kt in range(KT):
                        nc.tensor.matmul(
                            pu[:], usl(kt), xk(kt),
                            start=(kt == 0), stop=(kt == KT - 1),
                        )
                    sg = spool.tile([P, T], F32)
                    nc.scalar.activation(
                        sg[:], pg[:], mybir.ActivationFunctionType.Silu
                    )
                    nc.vector.tensor_mul(hT_sb[:, it, :], sg[:], pu[:])

            for mt in range(MT):
                ms = slice(mt * P, (mt + 1) * P)
                if mt < MT - 1:
                    py0 = psum.tile([P, 512], F32, tag="py0")
                    py1 = psum.tile([P, 512], F32, tag="py1")
                    for it in range(IT):
                        lhsT = hT_sb[:, it, ms]
                        nc.tensor.matmul(
                            py0[:], lhsT, wd_sb[:, it, 0:512],
                            start=(it == 0), stop=(it == IT - 1),
                        )
                        nc.tensor.matmul(
                            py1[:], lhsT, wd_sb[:, it, 512:1024],
                            start=(it == 0), stop=(it == IT - 1),
                        )
                    y0 = spool.tile([P, 512], F32, tag="y0")
                    nc.scalar.copy(y0[:], py0[:])
                    nc.sync.dma_start(y_d[ms, 0:512], y0[:])
                    y1 = spool.tile([P, 512], F32, tag="y1")
                    nc.vector.tensor_copy(y1[:], py1[:])
                    nc.sync.dma_start(y_d[ms, 512:1024], y1[:])
                else:
                    # last token tile: run the two 16-matmul chains
                    # back-to-back instead of interleaved, so the first
                    # half's copy+DMA overlaps the second half's matmuls and
                    # only one [128,512] copy+DMA remains after the last MM.
                    py0 = psum.tile([P, 512], F32, tag="py0")
                    for it in range(IT):
                        nc.tensor.matmul(
                            py0[:], hT_sb[:, it, ms], wd_sb[:, it, 0:512],
                            start=(it == 0), stop=(it == IT - 1),
                        )
                    y0 = spool.tile([P, 512], F32, tag="y0")
                    nc.scalar.copy(y0[:], py0[:])
                    nc.sync.dma_start(y_d[ms, 0:512], y0[:])
                    # split the remaining 512 columns into two N=256 chains
                    # (same total PE cycles) so the final exposed copy+DMA
                    # after the very last matmul is only [128,256]
                    py1a = psum.tile([P, 256], F32, tag="py1", name="py1a")
                    for it in range(IT):
                        nc.tensor.matmul(
                            py1a[:], hT_sb[:, it, ms], wd_sb[:, it, 512:768],
                            start=(it == 0), stop=(it == IT - 1),
                        )
                    y1 = spool.tile([P, 256], F32, tag="y1")
                    nc.scalar.copy(y1[:], py1a[:])
                    nc.sync.dma_start(y_d[ms, 512:768], y1[:])
                    py1b = psum.tile([P, 256], F32, tag="pu", name="py1b")
                    for it in range(IT):
                        nc.tensor.matmul(
                            py1b[:], hT_sb[:, it, ms], wd_sb[:, it, 768:1024],
                            start=(it == 0), stop=(it == IT - 1),
                        )
                    y2 = spool.tile([P, 256], F32, tag="y2")
                    nc.vector.tensor_copy(y2[:], py1b[:])
                    nc.sync.dma_start(y_d[ms, 768:1024], y2[:])

    nc.compile()
    return nc


def _get_compiled():
    global _compiled
    if _compiled is None:
        _compiled = _build()
    return _compiled


def _numpy_fallback(hidden_states, gate_kernel, up_kernel, down_kernel, group_sizes):
    # Exact reference math on host; only used for unexpected group_sizes.
    out = np.empty((hidden_states.shape[0], down_kernel.shape[2]), np.float32)
    start = 0
    for e in range(gate_kernel.shape[0]):
        g = int(group_sizes[e])
        x = hidden_states[start:start + g]
        gate = x @ gate_kernel[e]
        up = x @ up_kernel[e]
        sig = np.where(
            gate >= 0,
            1.0 / (1.0 + np.exp(-np.clip(gate, 0, None))),
            np.exp(np.clip(gate, None, 0))
            / (1.0 + np.exp(np.clip(gate, None, 0))),
        )
        h = gate * sig * up
        out[start:start + g] = h @ down_kernel[e]
        start += g
    out[start:] = 0.0
    return out


def _make_in_maps(hidden_states, gate_kernel, up_kernel, down_kernel):
    bf = ml_dtypes.bfloat16
    in_maps = []
    for e in range(E):
        x_e = hidden_states[e * T:(e + 1) * T]
        in_maps.append({
            "xT": np.ascontiguousarray(x_e.T).astype(bf),
            "wg": np.ascontiguousarray(gate_kernel[e]).astype(bf),
            "wu": np.ascontiguousarray(up_kernel[e]).astype(bf),
            "wd": np.ascontiguousarray(down_kernel[e]).astype(bf),
        })
    return in_maps


def profile_run(inputs, tmpdir=None):
    """Dev helper (not used by grading): run with NTFF tracing, return exec ns."""
    nc = _get_compiled()
    in_maps = _make_in_maps(
        np.asarray(inputs["hidden_states"], np.float32),
        np.asarray(inputs["gate_kernel"], np.float32),
        np.asarray(inputs["up_kernel"], np.float32),
        np.asarray(inputs["down_kernel"], np.float32),
    )
    res = run_bass_kernel_spmd(
        nc, in_maps, core_ids=list(range(E)), trace=True, tmpdir=tmpdir
    )
    return res.exec_time_ns


def kernel(hidden_states, gate_kernel, up_kernel, down_kernel, group_sizes):
    hidden_states = np.asarray(hidden_states, dtype=np.float32)
    gate_kernel = np.asarray(gate_kernel, dtype=np.float32)
    up_kernel = np.asarray(up_kernel, dtype=np.float32)
    down_kernel = np.asarray(down_kernel, dtype=np.float32)
    gs = np.asarray(group_sizes)

    if not (gs.shape == (E,) and np.all(gs == T)):
        return _numpy_fallback(
            hidden_states, gate_kernel, up_kernel, down_kernel, gs
        )

    nc = _get_compiled()
    in_maps = _make_in_maps(hidden_states, gate_kernel, up_kernel, down_kernel)
    res = run_bass_kernel_spmd(nc, in_maps, core_ids=list(range(E)))
    return np.concatenate([res.results[e]["y"] for e in range(E)], axis=0)
